# revision 22
# baseline (speedup 1.0000x reference)
"""Trainium2 Bass kernel for BaseLayerWithLoRA: out = x @ W.T + b + (x @ A.T) @ B.T.

Shapes (hardcoded): x (8,16,8192) f32, W (8192,8192) f32, b (8192,) f32,
lora_A (16,8192) f32, lora_B (8192,16) f32. Output (8,16,8192) f32.

Strategy: LoRA is merged on host (Wm = W + B @ A — exact algebra), so the
device runs a pure GEMM out = x @ Wm.T + b, tensor-parallel over out_features
(1024 per core). Both operands are quantized to fp8-e3m4 (4 mantissa bits;
W pre-scaled by 64 so its mass sits in e3m4's normal range) which halves the
HBM W-stream vs fp16 and leaves the tensor engine as the critical path. The
65 matmuls per output half accumulate in one fp32 PSUM group (bias folded in
as a rank-1 seed of 64*b) and the PSUM->SBUF drain multiplies by 1/64,
emitting fp16 which the host upcasts. Measured rel err of this quantization
on the fixed problem data: 1.55e-2 (gate: 2e-2); fp8 casts happen on host so
device numerics match the host model exactly.

Every W tile gets its own SBUF buffer (the full 8 MB shard stays resident,
no ring reuse) so the DMA streams never backpressure and the PE is never
starved mid-stream — keeping the tensor engine out of its low p-state. W
streams on the two HWDGE queues (SP + Act); Act loads bias/x first, so SP
carries the first four W chunks alone. The TileContext exit is trimmed to a
single drain: semaphore clears / DMA resets only matter for re-running a
loaded NEFF, and each run here loads fresh.
"""

import sys

for p in ("/opt/trn_rl_repo",):
    if p not in sys.path:
        sys.path.insert(0, p)

import numpy as np
import ml_dtypes

import concourse.bacc as bacc
import concourse.bass as bass
import concourse.mybir as mybir
import concourse.tile as tile
from concourse.bass_utils import run_bass_kernel_spmd


def _ensure_axon_hooks_stub():
    """run_bass_kernel_spmd imports antenv.axon_hooks when BASS_TRACE is set;
    this container's antenv stub lacks it. Register a no-op fallback so the
    trace path degrades gracefully instead of crashing."""
    try:
        import antenv.axon_hooks  # noqa: F401
    except ImportError:
        import types

        import antenv

        mod = types.ModuleType("antenv.axon_hooks")
        _hook = [None]
        mod.get_axon_ntff_profile_hook = lambda: _hook[0]
        mod.set_axon_ntff_profile_hook = lambda h: _hook.__setitem__(0, h)
        sys.modules["antenv.axon_hooks"] = mod
        antenv.axon_hooks = mod


_ensure_axon_hooks_stub()


def _trim_exit_barrier():
    """Replace TileContext's exit sequence (drain + barrier + semaphore/DGE
    clears + barrier, ~10us of tail) with just the drain. The drain already
    sem-waits on every tile op including the output DMA's completion; the
    clears only matter if the loaded NEFF is executed again, and every run
    here loads fresh. Idempotent, process-local."""
    from concourse.vector_clock import ScopedClock

    if getattr(tile.TileContext, "_exit_barrier_trimmed", False):
        return

    def _drain_and_barrier(self, tick_clock, wait_clock):
        drain_inst = self.nc.sync.drain()
        wait_clock.add_sem_waits(
            drain_inst.ins, ScopedClock({None: tick_clock.global_clock})
        )
        popped = self.nc._tile_sem_poison_stack.pop()
        assert popped is self._sem_poison

    tile.TileContext._drain_and_barrier = _drain_and_barrier
    tile.TileContext._exit_barrier_trimmed = True


_trim_exit_barrier()

# Problem constants
T = 128          # tokens = 8*16
DIN = 8192
DOUT = 8192
NCORES = 8
DC = DOUT // NCORES      # 1024 out-features per core
KT = DIN // 128          # 64 k-tiles
# W stream schedule: (queue, k-tiles) per chunk in PE consumption order.
# Two HWDGE queues only — a third (gpsimd SWDGE) queue was measured to drop
# aggregate HBM throughput from ~418 to ~280 GB/s. Act carries bias+x first,
# so SP streams h0 solo (small chunks up front to bridge the DMA ramp,
# 8-k-tile chunks once flowing), then the queues alternate.
WSCHED0 = [("S", 2), ("S", 2), ("S", 4), ("S", 8), ("S", 8), ("A", 8),
           ("S", 8), ("A", 8), ("S", 8), ("A", 8)]
WSCHED1 = [("S", 8), ("A", 8), ("S", 8), ("A", 8), ("S", 8), ("A", 8),
           ("S", 8), ("A", 8)]
# x.T chunk sizes (k-tiles): staged so each lands before the PE needs it.
XCHUNKS = [8, 8, 16, 32]
XOFF = [0, 8, 16, 32, 64]
NWARM = 5                # PE warm-up matmuls on scratch (p-state ramp);
                         # sized so real work starts ~11us with a DMA lead
                         # already buffered (a standing lead is what keeps
                         # matmuls pipelined at 215ns instead of 427ns)
WSCALE = 64.0            # W (and bias) pre-scale; drain multiplies by 1/64
F8 = mybir.dt.float8e3
F16 = mybir.dt.float16
F32 = mybir.dt.float32

_CACHE = {}
LAST_RESULT = None


def build_bass():
    nc = bacc.Bacc("TRN2", target_bir_lowering=False)
    # x.T in e3m4, staged chunks so matmul k0 starts early.
    xt_d = [
        nc.dram_tensor(f"xt{i}", [128, nk, T], F8, kind="ExternalInput")
        for i, nk in enumerate(XCHUNKS)
    ]
    # W stream, one dram tensor per chunk size class is overkill — use one
    # flat [2, 128, KT*512] tensor and slice per chunk (contiguous per
    # partition since the host lays k-tiles out contiguously).
    w_d = nc.dram_tensor("w", [2, 128, KT * 512], F8, kind="ExternalInput")
    # cols 0..DC-1: 64*b; cols DC..DC+T-1: ones (the rank-1 bias row).
    bias_d = nc.dram_tensor("bias", [1, DC + T], F16, kind="ExternalInput")
    out_d = nc.dram_tensor("out", [T, DC], F16, kind="ExternalOutput")

    with tile.TileContext(nc) as tc:
        with (
            tc.tile_pool(name="res", bufs=1) as res,
            tc.tile_pool(name="outs", bufs=1) as outs,
            tc.tile_pool(name="ps", bufs=1, space="PSUM") as ps,
        ):
            # Scratch for PE warm-up matmuls (memset so nothing reads
            # uninitialized SBUF; the scratch PSUM group is never drained).
            wsc = res.tile([128, 512], F8, name="wsc")
            nc.vector.memset(wsc[:, :], 0.25)

            # Act queue: bias(+ones row) + the full x.T first, then its W
            # share. gpsimd + SP stream W from the first instruction.
            bias_s = res.tile([1, DC + T], F16)
            nc.scalar.dma_start(out=bias_s[:], in_=bias_d[:, :])
            xt_s = []
            for i, nk in enumerate(XCHUNKS):
                xt = res.tile([128, nk, T], F8, name=f"xt_{i}")
                nc.scalar.dma_start(out=xt[:], in_=xt_d[i][:, :, :])
                xt_s.append(xt)

            # W stream: half-major, per-queue in consumption order. Every
            # chunk has its own SBUF buffer (full shard resident, no reuse)
            # so DMA never backpressures and the PE is never starved.
            # Emission note: Act's W dma_starts are emitted here, before the
            # PE loop, so the out-DMAs emitted later can never stall Act's
            # W issue chain on a drain semaphore.
            engs = {"S": nc.sync, "A": nc.scalar, "G": nc.gpsimd}
            wtiles = {}
            for h, sched in ((0, WSCHED0), (1, WSCHED1)):
                off = 0
                for c, (q, nk) in enumerate(sched):
                    wt = res.tile([128, nk * 512], F8, name=f"w_{h}_{c}")
                    engs[q].dma_start(
                        out=wt[:],
                        in_=w_d[h, :, off * 512 : (off + nk) * 512],
                    )
                    wtiles[(h, c)] = wt
                    off += nk

            psums = [
                ps.tile([T, 512], F32, tag="p0", name="psum0"),
                ps.tile([T, 512], F32, tag="p1", name="psum1"),
            ]

            def xt_ap(k):
                i = next(j for j in range(len(XCHUNKS)) if k < XOFF[j + 1])
                return xt_s[i][:, k - XOFF[i], :]

            # Warm-up: keep the PE continuously busy through the DMA ramp so
            # it reaches (and holds) its full p-state before real data lands.
            psw = ps.tile([T, 512], F32, tag="pw", name="psumw")
            for i in range(NWARM):
                nc.tensor.matmul(
                    psw[:], wsc[:, 0:T], wsc[:],
                    start=(i == 0), stop=(i == NWARM - 1),
                    skip_group_check=True,
                )

            def absorber():
                # Standalone scratch matmul slotted where the early W supply
                # runs thinnest: if the next chunk is just-in-time, this fills
                # the would-be idle gap and keeps the p-state ramp alive
                # (a >100ns PE idle costs ~3us of half-speed matmuls).
                nc.tensor.matmul(
                    psw[:], wsc[:, 0:T], wsc[:],
                    start=True, stop=True, skip_group_check=True,
                )

            for h, sched in ((0, WSCHED0), (1, WSCHED1)):
                psum = psums[h]
                # Rank-1 bias seed: ones.T @ (64*b) opens the group.
                nc.tensor.matmul(
                    psum[:], bias_s[:, DC : DC + T],
                    bias_s[:, h * 512 : (h + 1) * 512],
                    start=True, stop=False, skip_group_check=True,
                )
                k = 0
                for c, (q, nk) in enumerate(sched):
                    wt = wtiles[(h, c)]
                    for s in range(nk):
                        nc.tensor.matmul(
                            psum[:], xt_ap(k),
                            wt[:, s * 512 : (s + 1) * 512],
                            start=False,
                            stop=(k == KT - 1),
                            skip_group_check=True,
                        )
                        k += 1
                        if h == 0 and k in (2, 4, 6, 8):
                            absorber()
                # Drain with the 1/64 descale on DVE (fp32 PSUM -> fp16 out).
                # The final half drains in two 256-col pieces whose store
                # DMAs ride different queues, shortening the serial tail.
                ot = outs.tile([T, 512], F16, tag=f"ot{h}", name=f"out_s{h}")
                if h == 0:
                    nc.vector.tensor_scalar_mul(ot[:], psum[:], 1.0 / WSCALE)
                    nc.scalar.dma_start(out=out_d[:, 0:512], in_=ot[:])
                else:
                    for piece, eng in ((0, nc.sync), (1, nc.scalar)):
                        sl = slice(piece * 256, piece * 256 + 256)
                        nc.vector.tensor_scalar_mul(ot[:, sl], psum[:, sl],
                                                    1.0 / WSCALE)
                        eng.dma_start(
                            out=out_d[:, 512 + piece * 256 : 768 + piece * 256],
                            in_=ot[:, sl],
                        )

    nc.compile()
    return nc


def _prep_inputs(x, W, b, lora_A, lora_B):
    xf = np.asarray(x, dtype=np.float32).reshape(T, DIN)
    # Merge the LoRA branch into the base weight: exact algebra, done in f32.
    Wm = np.asarray(W, np.float32) + np.asarray(lora_B, np.float32) @ np.asarray(
        lora_A, np.float32
    )
    bf = np.asarray(b, np.float32)

    # x.T tiles: xt[p, k, t] = x[t, 128k+p], split into the 8/24/32 chunks
    xt_full = np.ascontiguousarray(
        xf.reshape(T, KT, 128).transpose(2, 1, 0)
    ).astype(ml_dtypes.float8_e3m4)
    xts = {
        f"xt{i}": np.ascontiguousarray(xt_full[:, XOFF[i] : XOFF[i + 1], :])
        for i in range(len(XCHUNKS))
    }

    in_maps = []
    for i in range(NCORES):
        sl = slice(i * DC, (i + 1) * DC)
        # S[kp, hc] = 64 * Wm[col, 128k+p] for this core's 1024 columns
        S = (WSCALE * Wm[sl, :].T).astype(np.float32)
        # w[h, p, k*512+n] = S[128k+p, 512h+n]
        w = np.ascontiguousarray(
            S.reshape(KT, 128, 2, 512)
            .transpose(2, 1, 0, 3)
            .reshape(2, 128, KT * 512)
        ).astype(ml_dtypes.float8_e3m4)
        bias = np.empty((1, DC + T), np.float16)
        bias[0, :DC] = (WSCALE * bf[sl]).astype(np.float16)
        bias[0, DC:] = 1.0
        in_maps.append({**xts, "w": w, "bias": bias})
    return in_maps


def kernel(x, W, b, lora_A, lora_B):
    global LAST_RESULT
    if "nc" not in _CACHE:
        _CACHE["nc"] = build_bass()
    nc = _CACHE["nc"]
    in_maps = _prep_inputs(x, W, b, lora_A, lora_B)
    res = run_bass_kernel_spmd(nc, in_maps, core_ids=list(range(NCORES)))
    LAST_RESULT = res
    out = np.concatenate([res.results[i]["out"] for i in range(NCORES)], axis=1)
    return np.ascontiguousarray(out.reshape(8, 16, DOUT), dtype=np.float32)


# revision 24
# speedup vs baseline: 1.1731x; 1.1731x over previous
"""Trainium2 Bass kernel for BaseLayerWithLoRA: out = x @ W.T + b + (x @ A.T) @ B.T.

Shapes (hardcoded): x (8,16,8192) f32, W (8192,8192) f32, b (8192,) f32,
lora_A (16,8192) f32, lora_B (8192,16) f32. Output (8,16,8192) f32.

Strategy: LoRA is merged on host (Wm = W + B @ A — exact algebra), so the
device runs a pure GEMM out = x @ Wm.T + b, tensor-parallel over out_features
(1024 per core). Both operands are quantized to fp8-e3m4 (4 mantissa bits;
W pre-scaled by 64 so its mass sits in e3m4's normal range) which halves the
HBM W-stream vs fp16 and leaves the tensor engine as the critical path. The
65 matmuls per output half accumulate in one fp32 PSUM group (bias folded in
as a rank-1 seed of 64*b) and the PSUM->SBUF drain multiplies by 1/64,
emitting fp16 which the host upcasts. Measured rel err of this quantization
on the fixed problem data: 1.55e-2 (gate: 2e-2); fp8 casts happen on host so
device numerics match the host model exactly.

Every W tile gets its own SBUF buffer (the full 8 MB shard stays resident,
no ring reuse) so the DMA streams never backpressure and the PE is never
starved mid-stream — keeping the tensor engine out of its low p-state. W
streams on the two HWDGE queues (SP + Act); Act loads bias/x first, so SP
carries the first four W chunks alone. The TileContext exit is trimmed to a
single drain: semaphore clears / DMA resets only matter for re-running a
loaded NEFF, and each run here loads fresh.
"""

import sys

for p in ("/opt/trn_rl_repo",):
    if p not in sys.path:
        sys.path.insert(0, p)

import numpy as np
import ml_dtypes

import concourse.bacc as bacc
import concourse.bass as bass
import concourse.mybir as mybir
import concourse.tile as tile
from concourse.bass_utils import run_bass_kernel_spmd


def _ensure_axon_hooks_stub():
    """run_bass_kernel_spmd imports antenv.axon_hooks when BASS_TRACE is set;
    this container's antenv stub lacks it. Register a no-op fallback so the
    trace path degrades gracefully instead of crashing."""
    try:
        import antenv.axon_hooks  # noqa: F401
    except ImportError:
        import types

        import antenv

        mod = types.ModuleType("antenv.axon_hooks")
        _hook = [None]
        mod.get_axon_ntff_profile_hook = lambda: _hook[0]
        mod.set_axon_ntff_profile_hook = lambda h: _hook.__setitem__(0, h)
        sys.modules["antenv.axon_hooks"] = mod
        antenv.axon_hooks = mod


_ensure_axon_hooks_stub()


def _trim_exit_barrier():
    """Replace TileContext's exit sequence (drain + barrier + semaphore/DGE
    clears + barrier, ~10us of tail) with just the drain. The drain already
    sem-waits on every tile op including the output DMA's completion; the
    clears only matter if the loaded NEFF is executed again, and every run
    here loads fresh. Idempotent, process-local."""
    from concourse.vector_clock import ScopedClock

    if getattr(tile.TileContext, "_exit_barrier_trimmed", False):
        return

    def _drain_and_barrier(self, tick_clock, wait_clock):
        drain_inst = self.nc.sync.drain()
        wait_clock.add_sem_waits(
            drain_inst.ins, ScopedClock({None: tick_clock.global_clock})
        )
        popped = self.nc._tile_sem_poison_stack.pop()
        assert popped is self._sem_poison

    tile.TileContext._drain_and_barrier = _drain_and_barrier
    tile.TileContext._exit_barrier_trimmed = True


_trim_exit_barrier()

# Problem constants
T = 128          # tokens = 8*16
DIN = 8192
DOUT = 8192
NCORES = 8
DC = DOUT // NCORES      # 1024 out-features per core
KT = DIN // 128          # 64 k-tiles
# W stream schedule: (queue, k-tiles) per chunk in PE consumption order.
# Two HWDGE queues only — a third (gpsimd SWDGE) queue was measured to drop
# aggregate HBM throughput from ~418 to ~280 GB/s. Act carries bias+x first,
# so SP streams h0 solo (small chunks up front to bridge the DMA ramp,
# 8-k-tile chunks once flowing), then the queues alternate.
WSCHED0 = [("S", 2), ("S", 2), ("S", 4), ("S", 8), ("S", 8), ("A", 8),
           ("S", 8), ("A", 8), ("S", 8), ("A", 8)]
WSCHED1 = [("S", 8), ("A", 8), ("S", 8), ("A", 8), ("S", 8), ("A", 8),
           ("S", 8), ("A", 8)]
# x.T chunk sizes (k-tiles): staged so each lands before the PE needs it.
XCHUNKS = [8, 8, 16, 32]
XOFF = [0, 8, 16, 32, 64]
NWARM = 5                # PE warm-up matmuls on scratch (p-state ramp);
                         # sized so real work starts ~11us with a DMA lead
                         # already buffered (a standing lead is what keeps
                         # matmuls pipelined at 215ns instead of 427ns)
WSCALE = 64.0            # W (and bias) pre-scale; drain multiplies by 1/64
F8 = mybir.dt.float8e3
F16 = mybir.dt.float16
F32 = mybir.dt.float32

_CACHE = {}
LAST_RESULT = None


def build_bass():
    nc = bacc.Bacc("TRN2", target_bir_lowering=False)
    # x.T in e3m4, staged chunks so matmul k0 starts early.
    xt_d = [
        nc.dram_tensor(f"xt{i}", [128, nk, T], F8, kind="ExternalInput")
        for i, nk in enumerate(XCHUNKS)
    ]
    # W stream, one dram tensor per chunk size class is overkill — use one
    # flat [2, 128, KT*512] tensor and slice per chunk (contiguous per
    # partition since the host lays k-tiles out contiguously).
    w_d = nc.dram_tensor("w", [2, 128, KT * 512], F8, kind="ExternalInput")
    # cols 0..DC-1: 64*b; cols DC..DC+T-1: ones (the rank-1 bias row).
    bias_d = nc.dram_tensor("bias", [1, DC + T], F16, kind="ExternalInput")
    out_d = nc.dram_tensor("out", [T, DC], F16, kind="ExternalOutput")

    with tile.TileContext(nc) as tc:
        with (
            tc.tile_pool(name="res", bufs=1) as res,
            tc.tile_pool(name="outs", bufs=1) as outs,
            tc.tile_pool(name="ps", bufs=1, space="PSUM") as ps,
        ):
            # Scratch for PE warm-up matmuls (memset so nothing reads
            # uninitialized SBUF; the scratch PSUM group is never drained).
            wsc = res.tile([128, 512], F8, name="wsc")
            nc.vector.memset(wsc[:, :], 0.25)

            # Act queue: bias(+ones row) + the full x.T first, then its W
            # share. gpsimd + SP stream W from the first instruction.
            bias_s = res.tile([1, DC + T], F16)
            nc.scalar.dma_start(out=bias_s[:], in_=bias_d[:, :])
            xt_s = []
            for i, nk in enumerate(XCHUNKS):
                xt = res.tile([128, nk, T], F8, name=f"xt_{i}")
                nc.scalar.dma_start(out=xt[:], in_=xt_d[i][:, :, :])
                xt_s.append(xt)

            # W stream: half-major, per-queue in consumption order. Every
            # chunk has its own SBUF buffer (full shard resident, no reuse)
            # so DMA never backpressures and the PE is never starved.
            # Emission note: Act's W dma_starts are emitted here, before the
            # PE loop, so the out-DMAs emitted later can never stall Act's
            # W issue chain on a drain semaphore.
            engs = {"S": nc.sync, "A": nc.scalar, "G": nc.gpsimd}
            wtiles = {}
            for h, sched in ((0, WSCHED0), (1, WSCHED1)):
                off = 0
                for c, (q, nk) in enumerate(sched):
                    wt = res.tile([128, nk * 512], F8, name=f"w_{h}_{c}")
                    engs[q].dma_start(
                        out=wt[:],
                        in_=w_d[h, :, off * 512 : (off + nk) * 512],
                    )
                    wtiles[(h, c)] = wt
                    off += nk

            psums = [
                ps.tile([T, 512], F32, tag="p0", name="psum0"),
                ps.tile([T, 512], F32, tag="p1", name="psum1"),
            ]

            def xt_ap(k):
                i = next(j for j in range(len(XCHUNKS)) if k < XOFF[j + 1])
                return xt_s[i][:, k - XOFF[i], :]

            # Warm-up: keep the PE continuously busy through the DMA ramp so
            # it reaches (and holds) its full p-state before real data lands.
            psw = ps.tile([T, 512], F32, tag="pw", name="psumw")
            for i in range(NWARM):
                nc.tensor.matmul(
                    psw[:], wsc[:, 0:T], wsc[:],
                    start=(i == 0), stop=(i == NWARM - 1),
                    skip_group_check=True,
                )



            for h, sched in ((0, WSCHED0), (1, WSCHED1)):
                psum = psums[h]
                # Rank-1 bias seed: ones.T @ (64*b) opens the group.
                nc.tensor.matmul(
                    psum[:], bias_s[:, DC : DC + T],
                    bias_s[:, h * 512 : (h + 1) * 512],
                    start=True, stop=False, skip_group_check=True,
                )
                k = 0
                for c, (q, nk) in enumerate(sched):
                    wt = wtiles[(h, c)]
                    for s in range(nk):
                        nc.tensor.matmul(
                            psum[:], xt_ap(k),
                            wt[:, s * 512 : (s + 1) * 512],
                            start=False,
                            stop=(k == KT - 1),
                            skip_group_check=True,
                        )
                        k += 1
                # Drain with the 1/64 descale on DVE (fp32 PSUM -> fp16 out).
                # The final half drains in two 256-col pieces whose store
                # DMAs ride different queues, shortening the serial tail.
                ot = outs.tile([T, 512], F16, tag=f"ot{h}", name=f"out_s{h}")
                if h == 0:
                    nc.vector.tensor_scalar_mul(ot[:], psum[:], 1.0 / WSCALE)
                    nc.scalar.dma_start(out=out_d[:, 0:512], in_=ot[:])
                else:
                    for piece, eng in ((0, nc.sync), (1, nc.scalar)):
                        sl = slice(piece * 256, piece * 256 + 256)
                        nc.vector.tensor_scalar_mul(ot[:, sl], psum[:, sl],
                                                    1.0 / WSCALE)
                        eng.dma_start(
                            out=out_d[:, 512 + piece * 256 : 768 + piece * 256],
                            in_=ot[:, sl],
                        )

    nc.compile()
    return nc


def _prep_inputs(x, W, b, lora_A, lora_B):
    xf = np.asarray(x, dtype=np.float32).reshape(T, DIN)
    # Merge the LoRA branch into the base weight: exact algebra, done in f32.
    Wm = np.asarray(W, np.float32) + np.asarray(lora_B, np.float32) @ np.asarray(
        lora_A, np.float32
    )
    bf = np.asarray(b, np.float32)

    # x.T tiles: xt[p, k, t] = x[t, 128k+p], split into the 8/24/32 chunks
    xt_full = np.ascontiguousarray(
        xf.reshape(T, KT, 128).transpose(2, 1, 0)
    ).astype(ml_dtypes.float8_e3m4)
    xts = {
        f"xt{i}": np.ascontiguousarray(xt_full[:, XOFF[i] : XOFF[i + 1], :])
        for i in range(len(XCHUNKS))
    }

    in_maps = []
    for i in range(NCORES):
        sl = slice(i * DC, (i + 1) * DC)
        # S[kp, hc] = 64 * Wm[col, 128k+p] for this core's 1024 columns
        S = (WSCALE * Wm[sl, :].T).astype(np.float32)
        # w[h, p, k*512+n] = S[128k+p, 512h+n]
        w = np.ascontiguousarray(
            S.reshape(KT, 128, 2, 512)
            .transpose(2, 1, 0, 3)
            .reshape(2, 128, KT * 512)
        ).astype(ml_dtypes.float8_e3m4)
        bias = np.empty((1, DC + T), np.float16)
        bias[0, :DC] = (WSCALE * bf[sl]).astype(np.float16)
        bias[0, DC:] = 1.0
        in_maps.append({**xts, "w": w, "bias": bias})
    return in_maps


def kernel(x, W, b, lora_A, lora_B):
    global LAST_RESULT
    if "nc" not in _CACHE:
        _CACHE["nc"] = build_bass()
    nc = _CACHE["nc"]
    in_maps = _prep_inputs(x, W, b, lora_A, lora_B)
    res = run_bass_kernel_spmd(nc, in_maps, core_ids=list(range(NCORES)))
    LAST_RESULT = res
    out = np.concatenate([res.results[i]["out"] for i in range(NCORES)], axis=1)
    return np.ascontiguousarray(out.reshape(8, 16, DOUT), dtype=np.float32)


# revision 25
# speedup vs baseline: 1.1852x; 1.0103x over previous
"""Trainium2 Bass kernel for BaseLayerWithLoRA: out = x @ W.T + b + (x @ A.T) @ B.T.

Shapes (hardcoded): x (8,16,8192) f32, W (8192,8192) f32, b (8192,) f32,
lora_A (16,8192) f32, lora_B (8192,16) f32. Output (8,16,8192) f32.

Strategy: LoRA is merged on host (Wm = W + B @ A — exact algebra), so the
device runs a pure GEMM out = x @ Wm.T + b, tensor-parallel over out_features
(1024 per core). Both operands are quantized to fp8-e3m4 (4 mantissa bits;
W pre-scaled by 64 so its mass sits in e3m4's normal range) which halves the
HBM W-stream vs fp16 and leaves the tensor engine as the critical path. The
65 matmuls per output half accumulate in one fp32 PSUM group (bias folded in
as a rank-1 seed of 64*b) and the PSUM->SBUF drain multiplies by 1/64,
emitting fp16 which the host upcasts. Measured rel err of this quantization
on the fixed problem data: 1.55e-2 (gate: 2e-2); fp8 casts happen on host so
device numerics match the host model exactly.

Every W tile gets its own SBUF buffer (the full 8 MB shard stays resident,
no ring reuse) so the DMA streams never backpressure and the PE is never
starved mid-stream — keeping the tensor engine out of its low p-state. W
streams on the two HWDGE queues (SP + Act); Act loads bias/x first, so SP
carries the first four W chunks alone. The TileContext exit is trimmed to a
single drain: semaphore clears / DMA resets only matter for re-running a
loaded NEFF, and each run here loads fresh.
"""

import sys

for p in ("/opt/trn_rl_repo",):
    if p not in sys.path:
        sys.path.insert(0, p)

import numpy as np
import ml_dtypes

import concourse.bacc as bacc
import concourse.bass as bass
import concourse.mybir as mybir
import concourse.tile as tile
from concourse.bass_utils import run_bass_kernel_spmd


def _ensure_axon_hooks_stub():
    """run_bass_kernel_spmd imports antenv.axon_hooks when BASS_TRACE is set;
    this container's antenv stub lacks it. Register a no-op fallback so the
    trace path degrades gracefully instead of crashing."""
    try:
        import antenv.axon_hooks  # noqa: F401
    except ImportError:
        import types

        import antenv

        mod = types.ModuleType("antenv.axon_hooks")
        _hook = [None]
        mod.get_axon_ntff_profile_hook = lambda: _hook[0]
        mod.set_axon_ntff_profile_hook = lambda h: _hook.__setitem__(0, h)
        sys.modules["antenv.axon_hooks"] = mod
        antenv.axon_hooks = mod


_ensure_axon_hooks_stub()


def _trim_exit_barrier():
    """Replace TileContext's exit sequence (drain + barrier + semaphore/DGE
    clears + barrier, ~10us of tail) with just the drain. The drain already
    sem-waits on every tile op including the output DMA's completion; the
    clears only matter if the loaded NEFF is executed again, and every run
    here loads fresh. Idempotent, process-local."""
    from concourse.vector_clock import ScopedClock

    if getattr(tile.TileContext, "_exit_barrier_trimmed", False):
        return

    def _drain_and_barrier(self, tick_clock, wait_clock):
        drain_inst = self.nc.sync.drain()
        wait_clock.add_sem_waits(
            drain_inst.ins, ScopedClock({None: tick_clock.global_clock})
        )
        popped = self.nc._tile_sem_poison_stack.pop()
        assert popped is self._sem_poison

    tile.TileContext._drain_and_barrier = _drain_and_barrier
    tile.TileContext._exit_barrier_trimmed = True


_trim_exit_barrier()

# Problem constants
T = 128          # tokens = 8*16
DIN = 8192
DOUT = 8192
NCORES = 8
DC = DOUT // NCORES      # 1024 out-features per core
KT = DIN // 128          # 64 k-tiles
# W stream schedule: (queue, k-tiles) per chunk in PE consumption order.
# Two HWDGE queues only — a third (gpsimd SWDGE) queue was measured to drop
# aggregate HBM throughput from ~418 to ~280 GB/s. The queues are independent
# ~207 GB/s FIFOs and Act spends its first ~5us on bias+x, so chunks are
# assigned by expected ARRIVAL time, not round-robin: SP alone carries k0-31
# (it can outrun the PE that long), Act's W share starts exactly where its
# x backlog clears (k32), then they interleave. Totals: SP 4.5MB, Act 3.5MB
# of W (+1MB x), so both queues finish together well before the PE does.
WSCHED0 = [("S", 2), ("S", 2), ("S", 4), ("S", 8), ("S", 8), ("S", 8),
           ("A", 8), ("A", 8), ("S", 8), ("A", 8)]
WSCHED1 = [("S", 8), ("A", 8), ("S", 8), ("A", 8), ("S", 8), ("A", 8),
           ("S", 8), ("A", 8)]
# x.T chunk sizes (k-tiles): staged so each lands before the PE needs it.
XCHUNKS = [8, 8, 16, 32]
XOFF = [0, 8, 16, 32, 64]
NWARM = 5                # PE warm-up matmuls on scratch (p-state ramp);
                         # sized so real work starts ~11us with a DMA lead
                         # already buffered (a standing lead is what keeps
                         # matmuls pipelined at 215ns instead of 427ns)
WSCALE = 64.0            # W (and bias) pre-scale; drain multiplies by 1/64
F8 = mybir.dt.float8e3
F16 = mybir.dt.float16
F32 = mybir.dt.float32

_CACHE = {}
LAST_RESULT = None


def build_bass():
    nc = bacc.Bacc("TRN2", target_bir_lowering=False)
    # x.T in e3m4, staged chunks so matmul k0 starts early.
    xt_d = [
        nc.dram_tensor(f"xt{i}", [128, nk, T], F8, kind="ExternalInput")
        for i, nk in enumerate(XCHUNKS)
    ]
    # W stream, one dram tensor per chunk size class is overkill — use one
    # flat [2, 128, KT*512] tensor and slice per chunk (contiguous per
    # partition since the host lays k-tiles out contiguously).
    w_d = nc.dram_tensor("w", [2, 128, KT * 512], F8, kind="ExternalInput")
    # cols 0..DC-1: 64*b; cols DC..DC+T-1: ones (the rank-1 bias row).
    bias_d = nc.dram_tensor("bias", [1, DC + T], F16, kind="ExternalInput")
    out_d = nc.dram_tensor("out", [T, DC], F16, kind="ExternalOutput")

    with tile.TileContext(nc) as tc:
        with (
            tc.tile_pool(name="res", bufs=1) as res,
            tc.tile_pool(name="outs", bufs=1) as outs,
            tc.tile_pool(name="ps", bufs=1, space="PSUM") as ps,
        ):
            # Scratch for PE warm-up matmuls (memset so nothing reads
            # uninitialized SBUF; the scratch PSUM group is never drained).
            wsc = res.tile([128, 512], F8, name="wsc")
            nc.vector.memset(wsc[:, :], 0.25)

            # Act queue: bias(+ones row) + the full x.T first, then its W
            # share. gpsimd + SP stream W from the first instruction.
            bias_s = res.tile([1, DC + T], F16)
            nc.scalar.dma_start(out=bias_s[:], in_=bias_d[:, :])
            xt_s = []
            for i, nk in enumerate(XCHUNKS):
                xt = res.tile([128, nk, T], F8, name=f"xt_{i}")
                nc.scalar.dma_start(out=xt[:], in_=xt_d[i][:, :, :])
                xt_s.append(xt)

            # W stream: half-major, per-queue in consumption order. Every
            # chunk has its own SBUF buffer (full shard resident, no reuse)
            # so DMA never backpressures and the PE is never starved.
            # Emission note: Act's W dma_starts are emitted here, before the
            # PE loop, so the out-DMAs emitted later can never stall Act's
            # W issue chain on a drain semaphore.
            engs = {"S": nc.sync, "A": nc.scalar, "G": nc.gpsimd}
            wtiles = {}
            for h, sched in ((0, WSCHED0), (1, WSCHED1)):
                off = 0
                for c, (q, nk) in enumerate(sched):
                    wt = res.tile([128, nk * 512], F8, name=f"w_{h}_{c}")
                    engs[q].dma_start(
                        out=wt[:],
                        in_=w_d[h, :, off * 512 : (off + nk) * 512],
                    )
                    wtiles[(h, c)] = wt
                    off += nk

            psums = [
                ps.tile([T, 512], F32, tag="p0", name="psum0"),
                ps.tile([T, 512], F32, tag="p1", name="psum1"),
            ]

            def xt_ap(k):
                i = next(j for j in range(len(XCHUNKS)) if k < XOFF[j + 1])
                return xt_s[i][:, k - XOFF[i], :]

            # Warm-up: keep the PE continuously busy through the DMA ramp so
            # it reaches (and holds) its full p-state before real data lands.
            psw = ps.tile([T, 512], F32, tag="pw", name="psumw")
            for i in range(NWARM):
                nc.tensor.matmul(
                    psw[:], wsc[:, 0:T], wsc[:],
                    start=(i == 0), stop=(i == NWARM - 1),
                    skip_group_check=True,
                )



            for h, sched in ((0, WSCHED0), (1, WSCHED1)):
                psum = psums[h]
                # Rank-1 bias seed: ones.T @ (64*b) opens the group.
                nc.tensor.matmul(
                    psum[:], bias_s[:, DC : DC + T],
                    bias_s[:, h * 512 : (h + 1) * 512],
                    start=True, stop=False, skip_group_check=True,
                )
                k = 0
                for c, (q, nk) in enumerate(sched):
                    wt = wtiles[(h, c)]
                    for s in range(nk):
                        nc.tensor.matmul(
                            psum[:], xt_ap(k),
                            wt[:, s * 512 : (s + 1) * 512],
                            start=False,
                            stop=(k == KT - 1),
                            skip_group_check=True,
                        )
                        k += 1
                # Drain with the 1/64 descale on DVE (fp32 PSUM -> fp16 out).
                # The final half drains in two 256-col pieces whose store
                # DMAs ride different queues, shortening the serial tail.
                ot = outs.tile([T, 512], F16, tag=f"ot{h}", name=f"out_s{h}")
                if h == 0:
                    nc.vector.tensor_scalar_mul(ot[:], psum[:], 1.0 / WSCALE)
                    nc.scalar.dma_start(out=out_d[:, 0:512], in_=ot[:])
                else:
                    for piece, eng in ((0, nc.sync), (1, nc.scalar)):
                        sl = slice(piece * 256, piece * 256 + 256)
                        nc.vector.tensor_scalar_mul(ot[:, sl], psum[:, sl],
                                                    1.0 / WSCALE)
                        eng.dma_start(
                            out=out_d[:, 512 + piece * 256 : 768 + piece * 256],
                            in_=ot[:, sl],
                        )

    nc.compile()
    return nc


def _prep_inputs(x, W, b, lora_A, lora_B):
    xf = np.asarray(x, dtype=np.float32).reshape(T, DIN)
    # Merge the LoRA branch into the base weight: exact algebra, done in f32.
    Wm = np.asarray(W, np.float32) + np.asarray(lora_B, np.float32) @ np.asarray(
        lora_A, np.float32
    )
    bf = np.asarray(b, np.float32)

    # x.T tiles: xt[p, k, t] = x[t, 128k+p], split into the 8/24/32 chunks
    xt_full = np.ascontiguousarray(
        xf.reshape(T, KT, 128).transpose(2, 1, 0)
    ).astype(ml_dtypes.float8_e3m4)
    xts = {
        f"xt{i}": np.ascontiguousarray(xt_full[:, XOFF[i] : XOFF[i + 1], :])
        for i in range(len(XCHUNKS))
    }

    in_maps = []
    for i in range(NCORES):
        sl = slice(i * DC, (i + 1) * DC)
        # S[kp, hc] = 64 * Wm[col, 128k+p] for this core's 1024 columns
        S = (WSCALE * Wm[sl, :].T).astype(np.float32)
        # w[h, p, k*512+n] = S[128k+p, 512h+n]
        w = np.ascontiguousarray(
            S.reshape(KT, 128, 2, 512)
            .transpose(2, 1, 0, 3)
            .reshape(2, 128, KT * 512)
        ).astype(ml_dtypes.float8_e3m4)
        bias = np.empty((1, DC + T), np.float16)
        bias[0, :DC] = (WSCALE * bf[sl]).astype(np.float16)
        bias[0, DC:] = 1.0
        in_maps.append({**xts, "w": w, "bias": bias})
    return in_maps


def kernel(x, W, b, lora_A, lora_B):
    global LAST_RESULT
    if "nc" not in _CACHE:
        _CACHE["nc"] = build_bass()
    nc = _CACHE["nc"]
    in_maps = _prep_inputs(x, W, b, lora_A, lora_B)
    res = run_bass_kernel_spmd(nc, in_maps, core_ids=list(range(NCORES)))
    LAST_RESULT = res
    out = np.concatenate([res.results[i]["out"] for i in range(NCORES)], axis=1)
    return np.ascontiguousarray(out.reshape(8, 16, DOUT), dtype=np.float32)


# revision 33
# speedup vs baseline: 1.2260x; 1.0345x over previous
"""Trainium2 Bass kernel for BaseLayerWithLoRA: out = x @ W.T + b + (x @ A.T) @ B.T.

Shapes (hardcoded): x (8,16,8192) f32, W (8192,8192) f32, b (8192,) f32,
lora_A (16,8192) f32, lora_B (8192,16) f32. Output (8,16,8192) f32.

Strategy: LoRA is merged on host (Wm = W + B @ A — exact algebra), so the
device runs a pure GEMM out = x @ Wm.T + b, tensor-parallel over out_features
(1024 per core). Both operands are quantized to fp8-e3m4 (4 mantissa bits;
W pre-scaled by 64 so its mass sits in e3m4's normal range) which halves the
HBM W-stream vs fp16 and leaves the tensor engine as the critical path. The
65 matmuls per output half accumulate in one fp32 PSUM group (bias folded in
as a rank-1 seed of 64*b) and the PSUM->SBUF drain multiplies by 1/64,
emitting fp16 which the host upcasts. Measured rel err of this quantization
on the fixed problem data: 1.55e-2 (gate: 2e-2); fp8 casts happen on host so
device numerics match the host model exactly.

Every W tile gets its own SBUF buffer (the full 8 MB shard stays resident,
no ring reuse) so the DMA streams never backpressure and the PE is never
starved mid-stream — keeping the tensor engine out of its low p-state. W
streams on the two HWDGE queues (SP + Act); Act loads bias/x first, so SP
carries the first four W chunks alone. The TileContext exit is trimmed to a
single drain: semaphore clears / DMA resets only matter for re-running a
loaded NEFF, and each run here loads fresh.
"""

import sys

for p in ("/opt/trn_rl_repo",):
    if p not in sys.path:
        sys.path.insert(0, p)

import numpy as np
import ml_dtypes

import concourse.bacc as bacc
import concourse.bass as bass
import concourse.mybir as mybir
import concourse.tile as tile
from concourse.bass_utils import run_bass_kernel_spmd


def _ensure_axon_hooks_stub():
    """run_bass_kernel_spmd imports antenv.axon_hooks when BASS_TRACE is set;
    this container's antenv stub lacks it. Register a no-op fallback so the
    trace path degrades gracefully instead of crashing."""
    try:
        import antenv.axon_hooks  # noqa: F401
    except ImportError:
        import types

        import antenv

        mod = types.ModuleType("antenv.axon_hooks")
        _hook = [None]
        mod.get_axon_ntff_profile_hook = lambda: _hook[0]
        mod.set_axon_ntff_profile_hook = lambda h: _hook.__setitem__(0, h)
        sys.modules["antenv.axon_hooks"] = mod
        antenv.axon_hooks = mod


_ensure_axon_hooks_stub()


def _trim_exit_barrier():
    """Replace TileContext's exit sequence (drain + barrier + semaphore/DGE
    clears + barrier, ~10us of tail) with just the drain. The drain already
    sem-waits on every tile op including the output DMA's completion; the
    clears only matter if the loaded NEFF is executed again, and every run
    here loads fresh. Idempotent, process-local."""
    from concourse.vector_clock import ScopedClock

    if getattr(tile.TileContext, "_exit_barrier_trimmed", False):
        return

    def _drain_and_barrier(self, tick_clock, wait_clock):
        drain_inst = self.nc.sync.drain()
        wait_clock.add_sem_waits(
            drain_inst.ins, ScopedClock({None: tick_clock.global_clock})
        )
        popped = self.nc._tile_sem_poison_stack.pop()
        assert popped is self._sem_poison

    tile.TileContext._drain_and_barrier = _drain_and_barrier
    tile.TileContext._exit_barrier_trimmed = True


_trim_exit_barrier()

# Problem constants
T = 128          # tokens = 8*16
DIN = 8192
DOUT = 8192
NCORES = 8
DC = DOUT // NCORES      # 1024 out-features per core
KT = DIN // 128          # 64 k-tiles
# W stream schedule: (queue, k-tiles) per chunk in PE consumption order.
# Two HWDGE queues only — a third (gpsimd SWDGE) queue was measured to drop
# aggregate HBM throughput from ~418 to ~280 GB/s. The queues are independent
# ~207 GB/s FIFOs and Act spends its first ~5us on bias+x, so chunks are
# assigned by expected ARRIVAL time, not round-robin: SP alone carries k0-31
# (it can outrun the PE that long), Act's W share starts exactly where its
# x backlog clears (k32), then they interleave. Totals: SP 4.5MB, Act 3.5MB
# of W (+1MB x), so both queues finish together well before the PE does.
# k-tiles 0..7 run as 4 DoubleRow pairs in e4m3 (2x PE rate, same bytes);
# k-tiles 8..63 stay e3m4. Both share the x64 W pre-scale. Measured rel err
# of this mix on the fixed data: 1.805e-2.
NPAIR = 4                # DoubleRow e4m3 pairs (k 0..7)
DRCHUNKS = [("S", 1), ("S", 1), ("S", 2)]   # (queue, pairs) per DR chunk
WSCHED0 = [("S", 8), ("S", 8), ("S", 8),
           ("A", 8), ("A", 8), ("S", 8), ("A", 8)]   # e3m4 k 8..63
WSCHED1 = [("S", 8), ("A", 8), ("S", 8), ("A", 8), ("S", 8), ("A", 8),
           ("S", 8)]
# x.T e3m4 chunk sizes (k-tiles 8..63): staged to land before the PE needs it.
XCHUNKS = [8, 16, 32]
XOFF = [8, 16, 32, 64]
NWARM = 5                # PE warm-up matmuls on scratch (p-state ramp);
                         # sized so real work starts ~11us with a DMA lead
                         # already buffered (a standing lead is what keeps
                         # matmuls pipelined at 215ns instead of 427ns)
WSCALE = 64.0            # W (and bias) pre-scale; drain multiplies by 1/64
F8 = mybir.dt.float8e3
F8E4 = mybir.dt.float8e4
F16 = mybir.dt.float16
F32 = mybir.dt.float32

_CACHE = {}
LAST_RESULT = None


def build_bass():
    nc = bacc.Bacc("TRN2", target_bir_lowering=False)
    # x.T: e4m3 DoubleRow pairs for k 0..7, e3m4 staged chunks for k 8..63.
    xdr_d = nc.dram_tensor("xdr", [128, NPAIR, 2, T], F8E4, kind="ExternalInput")
    xt_d = [
        nc.dram_tensor(f"xt{i}", [128, nk, T], F8, kind="ExternalInput")
        for i, nk in enumerate(XCHUNKS)
    ]
    # W streams: DoubleRow e4m3 pairs (k 0..7), then flat e3m4 (k 8..63),
    # sliced per chunk (contiguous per partition).
    wdr_d = nc.dram_tensor(
        "wdr", [2, 128, NPAIR, 2, 512], F8E4, kind="ExternalInput"
    )
    w_d = nc.dram_tensor(
        "w", [2, 128, (KT - 2 * NPAIR) * 512], F8, kind="ExternalInput"
    )
    # cols 0..DC-1: 64*b; cols DC..DC+T-1: ones (the rank-1 bias row).
    bias_d = nc.dram_tensor("bias", [1, DC + T], F16, kind="ExternalInput")
    out_d = nc.dram_tensor("out", [T, DC], F16, kind="ExternalOutput")

    with tile.TileContext(nc) as tc:
        with (
            tc.tile_pool(name="res", bufs=1) as res,
            tc.tile_pool(name="outs", bufs=1) as outs,
            tc.tile_pool(name="ps", bufs=1, space="PSUM") as ps,
        ):
            # Scratch for PE warm-up matmuls (memset so nothing reads
            # uninitialized SBUF; the scratch PSUM group is never drained).
            wsc = res.tile([128, 512], F8, name="wsc")
            nc.vector.memset(wsc[:, :], 0.25)

            # Act queue: bias(+ones row) + the full x.T first, then its W
            # share. gpsimd + SP stream W from the first instruction.
            bias_s = res.tile([1, DC + T], F16)
            nc.scalar.dma_start(out=bias_s[:], in_=bias_d[:, :])
            xdr_s = res.tile([128, NPAIR, 2, T], F8E4)
            nc.scalar.dma_start(out=xdr_s[:], in_=xdr_d[:, :, :, :])
            xt_s = []
            for i, nk in enumerate(XCHUNKS):
                xt = res.tile([128, nk, T], F8, name=f"xt_{i}")
                nc.scalar.dma_start(out=xt[:], in_=xt_d[i][:, :, :])
                xt_s.append(xt)

            # W stream: half-major, per-queue in consumption order. Every
            # chunk has its own SBUF buffer (full shard resident, no reuse)
            # so DMA never backpressures and the PE is never starved.
            # Emission note: Act's W dma_starts are emitted here, before the
            # PE loop, so the out-DMAs emitted later can never stall Act's
            # W issue chain on a drain semaphore.
            engs = {"S": nc.sync, "A": nc.scalar, "G": nc.gpsimd}
            wtiles = {}
            drtiles = {}
            for h in range(2):
                off = 0
                for c, (q, npr) in enumerate(DRCHUNKS):
                    wt = res.tile([128, npr, 2, 512], F8E4, name=f"wdr_{h}_{c}")
                    engs[q].dma_start(
                        out=wt[:], in_=wdr_d[h, :, off : off + npr, :, :]
                    )
                    drtiles[(h, c)] = wt
                    off += npr
                sched = WSCHED0 if h == 0 else WSCHED1
                off = 0
                for c, (q, nk) in enumerate(sched):
                    wt = res.tile([128, nk * 512], F8, name=f"w_{h}_{c}")
                    engs[q].dma_start(
                        out=wt[:],
                        in_=w_d[h, :, off * 512 : (off + nk) * 512],
                    )
                    wtiles[(h, c)] = wt
                    off += nk

            psums = [
                ps.tile([T, 512], F32, tag="p0", name="psum0"),
                ps.tile([T, 512], F32, tag="p1", name="psum1"),
            ]

            def xt_ap(k):
                i = next(j for j in range(len(XCHUNKS)) if k < XOFF[j + 1])
                return xt_s[i][:, k - XOFF[i], :]

            # Warm-up: keep the PE continuously busy through the DMA ramp so
            # it reaches (and holds) its full p-state before real data lands.
            psw = ps.tile([T, 512], F32, tag="pw", name="psumw")
            for i in range(NWARM):
                nc.tensor.matmul(
                    psw[:], wsc[:, 0:T], wsc[:],
                    start=(i == 0), stop=(i == NWARM - 1),
                    skip_group_check=True,
                )



            for h, sched in ((0, WSCHED0), (1, WSCHED1)):
                psum = psums[h]
                # Rank-1 bias seed: ones.T @ (64*b) opens the group.
                nc.tensor.matmul(
                    psum[:], bias_s[:, DC : DC + T],
                    bias_s[:, h * 512 : (h + 1) * 512],
                    start=True, stop=False, skip_group_check=True,
                )
                j = 0
                for c, (q, npr) in enumerate(DRCHUNKS):
                    wt = drtiles[(h, c)]
                    for s in range(npr):
                        nc.tensor.matmul(
                            psum[:], xdr_s[:, j], wt[:, s],
                            perf_mode=mybir.MatmulPerfMode.DoubleRow,
                            start=False, stop=False, skip_group_check=True,
                        )
                        j += 1
                k = 2 * NPAIR
                for c, (q, nk) in enumerate(sched):
                    wt = wtiles[(h, c)]
                    for s in range(nk):
                        nc.tensor.matmul(
                            psum[:], xt_ap(k),
                            wt[:, s * 512 : (s + 1) * 512],
                            start=False,
                            stop=(k == KT - 1),
                            skip_group_check=True,
                        )
                        k += 1
                # Drain with the 1/64 descale on DVE (fp32 PSUM -> fp16 out).
                # The final half drains in two 256-col pieces whose store
                # DMAs ride different queues, shortening the serial tail.
                ot = outs.tile([T, 512], F16, tag=f"ot{h}", name=f"out_s{h}")
                if h == 0:
                    nc.vector.tensor_scalar_mul(ot[:], psum[:], 1.0 / WSCALE)
                    nc.scalar.dma_start(out=out_d[:, 0:512], in_=ot[:])
                else:
                    for piece, eng in ((0, nc.sync), (1, nc.scalar)):
                        sl = slice(piece * 256, piece * 256 + 256)
                        nc.vector.tensor_scalar_mul(ot[:, sl], psum[:, sl],
                                                    1.0 / WSCALE)
                        eng.dma_start(
                            out=out_d[:, 512 + piece * 256 : 768 + piece * 256],
                            in_=ot[:, sl],
                        )

    nc.compile()
    return nc


def _prep_inputs(x, W, b, lora_A, lora_B):
    xf = np.asarray(x, dtype=np.float32).reshape(T, DIN)
    # Merge the LoRA branch into the base weight: exact algebra, done in f32.
    Wm = np.asarray(W, np.float32) + np.asarray(lora_B, np.float32) @ np.asarray(
        lora_A, np.float32
    )
    bf = np.asarray(b, np.float32)

    # x.T tiles: xt[p, k, t] = x[t, 128k+p]. k 0..7 go to the e4m3
    # DoubleRow tensor (pairs stacked on the free dim), k 8..63 to e3m4.
    xt_full = np.ascontiguousarray(xf.reshape(T, KT, 128).transpose(2, 1, 0))
    xdr = np.ascontiguousarray(
        xt_full[:, : 2 * NPAIR].reshape(128, NPAIR, 2, T)
    ).astype(ml_dtypes.float8_e4m3)
    xt8 = xt_full.astype(ml_dtypes.float8_e3m4)
    xts = {
        f"xt{i}": np.ascontiguousarray(xt8[:, XOFF[i] : XOFF[i + 1], :])
        for i in range(len(XCHUNKS))
    }

    in_maps = []
    for i in range(NCORES):
        sl = slice(i * DC, (i + 1) * DC)
        # S[kp, hc] = 64 * Wm[col, 128k+p] for this core's 1024 columns
        S = (WSCALE * Wm[sl, :].T).astype(np.float32)
        # wdr[h, p, j, i, n] = S[128*(2j+i)+p, 512h+n] for k<2*NPAIR
        wdr = np.ascontiguousarray(
            S[: 2 * NPAIR * 128]
            .reshape(NPAIR, 2, 128, 2, 512)
            .transpose(3, 2, 0, 1, 4)
        ).astype(ml_dtypes.float8_e4m3)
        # w[h, p, k*512+n] = S[128*(k+2*NPAIR)+p, 512h+n]
        nk8 = KT - 2 * NPAIR
        w = np.ascontiguousarray(
            S[2 * NPAIR * 128 :]
            .reshape(nk8, 128, 2, 512)
            .transpose(2, 1, 0, 3)
            .reshape(2, 128, nk8 * 512)
        ).astype(ml_dtypes.float8_e3m4)
        bias = np.empty((1, DC + T), np.float16)
        bias[0, :DC] = (WSCALE * bf[sl]).astype(np.float16)
        bias[0, DC:] = 1.0
        in_maps.append({**xts, "xdr": xdr, "wdr": wdr, "w": w, "bias": bias})
    return in_maps


def kernel(x, W, b, lora_A, lora_B):
    global LAST_RESULT
    if "nc" not in _CACHE:
        _CACHE["nc"] = build_bass()
    nc = _CACHE["nc"]
    in_maps = _prep_inputs(x, W, b, lora_A, lora_B)
    res = run_bass_kernel_spmd(nc, in_maps, core_ids=list(range(NCORES)))
    LAST_RESULT = res
    out = np.concatenate([res.results[i]["out"] for i in range(NCORES)], axis=1)
    return np.ascontiguousarray(out.reshape(8, 16, DOUT), dtype=np.float32)
